# revision 8
# baseline (speedup 1.0000x reference)
"""JointLoss (YOLO-style bbox + landmarks + confidence) on 8 Trainium2 cores.

Strategy: the three losses only read predictions at obj cells (<= B*T = 1024
of the 207360 grid cells) except the confidence term, which needs
sum(conf^2) over the whole grid.  Host builds the target assignment (tiny:
32x32 IoU argmax + scatter, replicated bit-exactly with jax-CPU), gathers
the obj-cell rows, and ships per-core: the core's bbox-prediction slab (for
the dense conf reduction) + gathered rows packed into one tensor.  Device
(data-parallel over batch, 4 batches/core) computes per-partition partial
sums; host combines in f64.

Raw Bass (no TileContext: its multi-wait tail drain does not compile on
this walrus build).  Explicit semaphores; DVE write-buffer drains between
dependent op levels (raw Bass does not auto-insert them).
"""

import numpy as np

B, T, G, A = 32, 32, 36, 5
NCORES = 8
BPC = B // NCORES            # batches per core
CELLS = G * G * A            # 6480 per batch
ROWS = BPC * T               # max obj rows per core = 128
SLAB_P, SLAB_F = 120, 216    # 120 * 216 * 5 == BPC * CELLS * 5 == 129600
SMALL_F = 284                # 136 lmp + 136 lmt + 4 bbp + 4 bbt + 4 aux

IMAGE_SIZE = 288.0
ANCHORS = np.array([[0.24, 0.24], [0.12, 0.12], [0.08, 0.08],
                    [0.28, 0.28], [0.15, 0.15]], dtype=np.float32)

_STATE = {}


def _build_program():
    import concourse.bass as bass
    from concourse import mybir
    from contextlib import ExitStack

    nc = bass.Bass()
    f32 = mybir.dt.float32
    small_p = nc.declare_dram_parameter("small", [ROWS, SMALL_F], f32, isOutput=False)
    slab_p = nc.declare_dram_parameter("slab", [SLAB_P, SLAB_F, 5], f32, isOutput=False)
    out_p = nc.declare_dram_parameter("out", [ROWS, 8], f32, isOutput=True)

    st = ExitStack()
    Tt = lambda n, s: st.enter_context(nc.sbuf_tensor(n, s, f32))
    small_t = Tt("small_t", [ROWS, SMALL_F])
    slab_t = Tt("slab_t", [SLAB_P, SLAB_F, 5])
    slabsq = Tt("slabsq", [SLAB_P, SLAB_F, 1])
    ldiff = Tt("ldiff", [ROWS, 68, 2])
    lsq = Tt("lsq", [ROWS, 68, 2])
    pairsum = Tt("pairsum", [ROWS, 68, 1])
    d_t = Tt("d_t", [ROWS, 68, 1])
    s_t = Tt("s_t", [ROWS, 1])
    bdiff = Tt("bdiff", [ROWS, 4])
    bneg = Tt("bneg", [ROWS, 4])
    bad = Tt("bad", [ROWS, 4])
    bt_ = Tt("bt_", [ROWS, 4])
    bth = Tt("bth", [ROWS, 4])
    bu = Tt("bu", [ROWS, 4])
    bsl = Tt("bsl", [ROWS, 4])
    ones4 = Tt("ones4", [ROWS, 4])
    negh4 = Tt("negh4", [ROWS, 4])
    zero4 = Tt("zero4", [ROWS, 4])
    cm1 = Tt("cm1", [ROWS, 1])
    cm1sq = Tt("cm1sq", [ROWS, 1])
    csq_ = Tt("csq_", [ROWS, 1])
    outtile = Tt("outtile", [ROWS, 8])

    lmp_v = small_t[:, 0:136]
    lmt_v = small_t[:, 136:272]
    bbp_v = small_t[:, 272:276]
    bbt_v = small_t[:, 276:280]
    aux0 = small_t[:, 280:281]   # gathered conf
    aux1 = small_t[:, 281:282]   # mask / nf
    aux2 = small_t[:, 282:283]   # mask

    op = mybir.AluOpType
    ax = mybir.AxisListType

    with nc.Block() as block, \
            nc.semaphore("dsem") as dsem, \
            nc.semaphore("vsem") as vsem, \
            nc.semaphore("asem") as asem, \
            nc.semaphore("csem") as csem, \
            nc.semaphore("osem") as osem:

        @block.sync
        def _(sync):
            sync.dma_start(out=small_t[:], in_=small_p[:]).then_inc(dsem, 16)
            sync.dma_start(out=slab_t[:], in_=slab_p[:]).then_inc(dsem, 16)
            sync.wait_ge(csem, 1)
            sync.dma_start(out=out_p[:], in_=outtile[:]).then_inc(osem, 16)
            sync.wait_ge(osem, 16)

        @block.vector
        def _(vector):
            vector.memset(outtile[:], 0.0)
            vector.memset(ones4[:], 1.0)
            vector.memset(negh4[:], -0.5)
            vector.memset(zero4[:], 0.0)
            vector.drain()
            vector.wait_ge(dsem, 16)        # small rows landed
            # L1
            vector.tensor_tensor(out=ldiff[:], in0=lmp_v, in1=lmt_v, op=op.subtract)
            vector.tensor_tensor(out=bdiff[:], in0=bbp_v, in1=bbt_v, op=op.subtract)
            vector.tensor_tensor(out=cm1[:], in0=aux0, in1=ones4[:, 0:1], op=op.subtract)
            vector.tensor_tensor(out=csq_[:], in0=aux0, in1=aux0, op=op.mult)
            vector.drain()
            # L2
            vector.tensor_mul(lsq[:], ldiff[:], ldiff[:])
            vector.tensor_sub(bneg[:], zero4[:], bdiff[:])
            vector.tensor_mul(cm1sq[:], cm1[:], cm1[:])
            vector.tensor_mul(outtile[:, 4:5], csq_[:], aux2)
            vector.drain()
            # L3
            vector.tensor_tensor(out=pairsum[:], in0=lsq[:, :, 0:1], in1=lsq[:, :, 1:2], op=op.add)
            vector.tensor_tensor(out=bad[:], in0=bdiff[:], in1=bneg[:], op=op.max)
            vector.tensor_mul(outtile[:, 3:4], cm1sq[:], aux2)
            vector.drain().then_inc(vsem, 1)      # ACT may start sqrt
            # smooth-L1 tail: t=min(|d|,1); sl1 = t*(|d| - 0.5 t)
            vector.tensor_tensor(out=bt_[:], in0=bad[:], in1=ones4[:], op=op.min)
            vector.drain()
            vector.tensor_mul(bth[:], bt_[:], negh4[:])
            vector.drain()
            vector.tensor_add(bu[:], bad[:], bth[:])
            vector.drain()
            vector.tensor_mul(bsl[:], bt_[:], bu[:])
            vector.drain()
            vector.tensor_reduce(out=outtile[:, 2:3], in_=bsl[:], axis=ax.X, op=op.add)
            # dense conf^2 over this core's slab (channel 4 of 5, stride-5 read)
            vector.wait_ge(dsem, 32)        # slab landed (overlapped with the above)
            vector.tensor_mul(slabsq[:], slab_t[:, :, 4:5], slab_t[:, :, 4:5])
            vector.drain()
            vector.tensor_reduce(out=outtile[0:SLAB_P, 0:1], in_=slabsq[:], axis=ax.XY, op=op.add)
            # nme = s * (mask/nf)
            vector.wait_ge(asem, 1)
            vector.tensor_mul(outtile[:, 1:2], s_t[:], aux1)
            vector.drain().then_inc(csem, 1)

        @block.scalar
        def _(scalar):
            scalar.wait_ge(vsem, 1)
            scalar.activation(
                out=d_t[:], in_=pairsum[:],
                func=mybir.ActivationFunctionType.Sqrt, accum_out=s_t[:],
            )
            scalar.drain().then_inc(asem, 1)

    st.close()
    return nc


def _get_nc():
    if "nc" not in _STATE:
        _STATE["nc"] = _build_program()
    return _STATE["nc"]


def _build_targets_host(bbox_target):
    """Replicate reference build_targets' cell assignment exactly (jax-CPU),
    returning the winning target index per grid cell (-1 = no object)."""
    import jax
    import jax.numpy as jnp

    cpu = jax.devices("cpu")[0]
    with jax.default_device(cpu):
        bt = jnp.asarray(np.asarray(bbox_target), dtype=jnp.float32)
        gt = bt[..., :4]
        valid = jnp.sum(bt, axis=-1) != 0
        gi = (gt[..., 0] * G).astype(jnp.int32)
        gj = (gt[..., 1] * G).astype(jnp.int32)
        acx = (0.5 + gi.astype(gt.dtype)) / G
        acy = (0.5 + gj.astype(gt.dtype)) / G
        aw = jnp.asarray(ANCHORS)[:, 0]
        ah = jnp.asarray(ANCHORS)[:, 1]

        def corners(cx, cy, w, h):
            x1 = (cx - w / 2) * IMAGE_SIZE
            x2 = (cx + w / 2) * IMAGE_SIZE
            y1 = (cy - h / 2) * IMAGE_SIZE
            y2 = (cy + h / 2) * IMAGE_SIZE
            return x1, x2, y1, y2

        gx1, gx2, gy1, gy2 = corners(gt[..., 0], gt[..., 1], gt[..., 2], gt[..., 3])
        ax1, ax2, ay1, ay2 = corners(acx[..., None], acy[..., None], aw, ah)
        ix1 = jnp.maximum(gx1[..., None], ax1)
        iy1 = jnp.maximum(gy1[..., None], ay1)
        ix2 = jnp.minimum(gx2[..., None], ax2)
        iy2 = jnp.minimum(gy2[..., None], ay2)
        inter = (ix2 - ix1 + 1) * (iy2 - iy1 + 1)
        area_g = ((gx2 - gx1 + 1) * (gy2 - gy1 + 1))[..., None]
        area_a = (ax2 - ax1 + 1) * (ay2 - ay1 + 1)
        iou = inter / (area_g + area_a - inter + 1e-16)
        best = jnp.argmax(iou, axis=-1)
        b_idx = jnp.broadcast_to(jnp.arange(B)[:, None], (B, T))
        gj_s = jnp.where(valid, gj, G)
        tnum = jnp.broadcast_to(jnp.arange(T)[None, :], (B, T))
        win = (
            jnp.full((B, G, G, A), -1, jnp.int32)
            .at[b_idx, gj_s, gi, best]
            .set(tnum, mode="drop")
        )
    return np.asarray(win)


def _prepare(bbox_prediction, landmarks_prediction, bbox_target, landmarks_target):
    """Host prep: target assignment + gather.  Returns (in_maps, n_obj)."""
    bbox_prediction = np.asarray(bbox_prediction, dtype=np.float32)
    landmarks_prediction = np.asarray(landmarks_prediction, dtype=np.float32)
    bbox_target = np.asarray(bbox_target, dtype=np.float32)
    landmarks_target = np.asarray(landmarks_target, dtype=np.float32)

    win = _build_targets_host(bbox_target)
    cells = np.argwhere(win >= 0)                      # (n, 4): b, gj, gi, a
    twin = win[win >= 0]                               # aligned winners
    n_obj = len(cells)

    cb, cj, ci, ca = cells[:, 0], cells[:, 1], cells[:, 2], cells[:, 3]
    lmp_all = landmarks_prediction[cb, cj, ci, ca].reshape(n_obj, 136)
    lmt_all = landmarks_target[cb, twin].reshape(n_obj, 136)
    bbp_all = bbox_prediction[cb, cj, ci, ca, :4]      # (n, 4)
    bbt_all = np.log1p(bbox_target[cb, twin, :4]).astype(np.float32)
    conf_all = bbox_prediction[cb, cj, ci, ca, 4]      # (n,)
    nf_all = np.sqrt(bbt_all[:, 2] * bbt_all[:, 3]).astype(np.float32)
    w_all = (np.float32(1.0) / nf_all).astype(np.float32)

    in_maps = []
    for c in range(NCORES):
        sel = (cb >= c * BPC) & (cb < (c + 1) * BPC)
        r = int(sel.sum())
        small = np.zeros((ROWS, SMALL_F), np.float32)
        small[:r, 0:136] = lmp_all[sel]
        small[:r, 136:272] = lmt_all[sel]
        small[:r, 272:276] = bbp_all[sel]
        small[:r, 276:280] = bbt_all[sel]
        small[:r, 280] = conf_all[sel]
        small[:r, 281] = w_all[sel]
        small[:r, 282] = 1.0
        slab = np.ascontiguousarray(
            bbox_prediction[c * BPC:(c + 1) * BPC].reshape(SLAB_P, SLAB_F, 5))
        in_maps.append({"small": small, "slab": slab})
    return in_maps, n_obj


def _combine(results, n_obj):
    S = np.zeros(5, np.float64)
    for r in results:
        o = r["out"].astype(np.float64)
        S += o[:, :5].sum(axis=0)
    s_slab, s_nme, s_loc, s_cse, s_csq = S
    n_obj_c = max(float(n_obj), 1.0)
    n_noobj = max(float(B * CELLS - n_obj), 1.0)
    nme = 2.0 * s_nme / (68.0 * n_obj_c)
    loc = 5.0 * s_loc / (n_obj_c * 4.0)
    conf = 0.5 * (s_slab - s_csq) / n_noobj + s_cse / n_obj_c
    return (np.float32(nme), np.float32(loc), np.float32(conf))


def _run_device(in_maps, trace=False):
    from concourse.bass_utils import run_bass_kernel_spmd
    nc = _get_nc()
    return run_bass_kernel_spmd(nc, in_maps, list(range(NCORES)), trace=trace)


def kernel(bbox_prediction, landmarks_prediction, bbox_target, landmarks_target):
    in_maps, n_obj = _prepare(
        bbox_prediction, landmarks_prediction, bbox_target, landmarks_target)
    res = _run_device(in_maps)
    return _combine(res.results, n_obj)


# revision 11
# speedup vs baseline: 1.0128x; 1.0128x over previous
"""JointLoss (YOLO-style bbox + landmarks + confidence) on 8 Trainium2 cores.

Strategy: the three losses only read predictions at obj cells (<= B*T = 1024
of the 207360 grid cells) except the confidence term, which needs
sum(conf^2) over the whole grid.  Host builds the target assignment (tiny:
32x32 IoU argmax + scatter, replicated bit-exactly with jax-CPU), gathers
the obj-cell rows, and ships per-core: the core's bbox-prediction slab (for
the dense conf reduction) + gathered rows packed into one tensor.  Device
(data-parallel over batch, 4 batches/core) computes per-partition partial
sums; host combines in f64.

Raw Bass (no TileContext: its multi-wait tail drain does not compile on
this walrus build).  Explicit semaphores; DVE write-buffer drains between
dependent op levels (raw Bass does not auto-insert them).
"""

import numpy as np

B, T, G, A = 32, 32, 36, 5
NCORES = 8
BPC = B // NCORES            # batches per core
CELLS = G * G * A            # 6480 per batch
ROWS = BPC * T               # max obj rows per core = 128
SLAB_P, SLAB_F = 120, 216    # 120 * 216 * 5 == BPC * CELLS * 5 == 129600
SMALL_F = 284                # 136 lmp + 136 lmt + 4 bbp + 4 bbt + 4 aux

IMAGE_SIZE = 288.0
ANCHORS = np.array([[0.24, 0.24], [0.12, 0.12], [0.08, 0.08],
                    [0.28, 0.28], [0.15, 0.15]], dtype=np.float32)

_STATE = {}


def _build_program():
    import concourse.bass as bass
    from concourse import mybir
    from contextlib import ExitStack

    nc = bass.Bass()
    f32 = mybir.dt.float32
    small_p = nc.declare_dram_parameter("small", [ROWS, SMALL_F], f32, isOutput=False)
    slab_p = nc.declare_dram_parameter("slab", [SLAB_P, SLAB_F, 5], f32, isOutput=False)
    out_p = nc.declare_dram_parameter("out", [ROWS, 8], f32, isOutput=True)

    st = ExitStack()
    Tt = lambda n, s: st.enter_context(nc.sbuf_tensor(n, s, f32))
    small_t = Tt("small_t", [ROWS, SMALL_F])
    slab_t = Tt("slab_t", [SLAB_P, SLAB_F, 5])
    slabjunk = Tt("slabjunk", [SLAB_P, SLAB_F, 1])
    ldiff = Tt("ldiff", [ROWS, 68, 2])
    lsq = Tt("lsq", [ROWS, 68, 2])
    pairsum = Tt("pairsum", [ROWS, 68, 1])
    d_t = Tt("d_t", [ROWS, 68, 1])
    s_t = Tt("s_t", [ROWS, 1])
    bdiff = Tt("bdiff", [ROWS, 4])
    bneg = Tt("bneg", [ROWS, 4])
    bad = Tt("bad", [ROWS, 4])
    bt_ = Tt("bt_", [ROWS, 4])
    bth = Tt("bth", [ROWS, 4])
    bu = Tt("bu", [ROWS, 4])
    bsl = Tt("bsl", [ROWS, 4])
    ones4 = Tt("ones4", [ROWS, 4])
    negh4 = Tt("negh4", [ROWS, 4])
    zero4 = Tt("zero4", [ROWS, 4])
    cm1 = Tt("cm1", [ROWS, 1])
    cm1sq = Tt("cm1sq", [ROWS, 1])
    csq_ = Tt("csq_", [ROWS, 1])
    outtile = Tt("outtile", [ROWS, 8])

    lmp_v = small_t[:, 0:136]
    lmt_v = small_t[:, 136:272]
    bbp_v = small_t[:, 272:276]
    bbt_v = small_t[:, 276:280]
    aux0 = small_t[:, 280:281]   # gathered conf
    aux1 = small_t[:, 281:282]   # mask / nf
    aux2 = small_t[:, 282:283]   # mask

    op = mybir.AluOpType
    ax = mybir.AxisListType

    with nc.Block() as block, \
            nc.semaphore("dsem") as dsem, \
            nc.semaphore("vsem") as vsem, \
            nc.semaphore("asem") as asem, \
            nc.semaphore("csem") as csem, \
            nc.semaphore("osem") as osem:

        @block.sync
        def _(sync):
            sync.dma_start(out=small_t[:], in_=small_p[:]).then_inc(dsem, 16)
            sync.dma_start(out=slab_t[:], in_=slab_p[:]).then_inc(dsem, 16)
            sync.wait_ge(csem, 2)
            sync.dma_start(out=out_p[:], in_=outtile[:]).then_inc(osem, 16)
            sync.wait_ge(osem, 16)

        @block.vector
        def _(vector):
            vector.memset(outtile[:], 0.0)
            vector.memset(ones4[:], 1.0)
            vector.memset(negh4[:], -0.5)
            vector.memset(zero4[:], 0.0)
            vector.drain()
            vector.wait_ge(dsem, 16)        # small rows landed
            # L1
            vector.tensor_tensor(out=ldiff[:], in0=lmp_v, in1=lmt_v, op=op.subtract)
            vector.tensor_tensor(out=bdiff[:], in0=bbp_v, in1=bbt_v, op=op.subtract)
            vector.tensor_tensor(out=cm1[:], in0=aux0, in1=ones4[:, 0:1], op=op.subtract)
            vector.tensor_tensor(out=csq_[:], in0=aux0, in1=aux0, op=op.mult)
            vector.drain()
            # L2
            vector.tensor_mul(lsq[:], ldiff[:], ldiff[:])
            vector.tensor_sub(bneg[:], zero4[:], bdiff[:])
            vector.tensor_mul(cm1sq[:], cm1[:], cm1[:])
            vector.tensor_mul(outtile[:, 4:5], csq_[:], aux2)
            vector.drain()
            # L3
            vector.tensor_tensor(out=pairsum[:], in0=lsq[:, :, 0:1], in1=lsq[:, :, 1:2], op=op.add)
            vector.tensor_tensor(out=bad[:], in0=bdiff[:], in1=bneg[:], op=op.max)
            vector.tensor_mul(outtile[:, 3:4], cm1sq[:], aux2)
            vector.drain().then_inc(vsem, 1)      # ACT may start sqrt
            # smooth-L1 tail: t=min(|d|,1); sl1 = t*(|d| - 0.5 t)
            vector.tensor_tensor(out=bt_[:], in0=bad[:], in1=ones4[:], op=op.min)
            vector.drain()
            vector.tensor_mul(bth[:], bt_[:], negh4[:])
            vector.drain()
            vector.tensor_add(bu[:], bad[:], bth[:])
            vector.drain()
            vector.tensor_mul(bsl[:], bt_[:], bu[:])
            vector.drain()
            vector.tensor_reduce(out=outtile[:, 2:3], in_=bsl[:], axis=ax.X, op=op.add)
            # nme = s * (mask/nf)
            vector.wait_ge(asem, 1)
            vector.tensor_mul(outtile[:, 1:2], s_t[:], aux1)
            vector.drain().then_inc(csem, 1)

        @block.scalar
        def _(scalar):
            scalar.wait_ge(vsem, 1)
            scalar.activation(
                out=d_t[:], in_=pairsum[:],
                func=mybir.ActivationFunctionType.Sqrt, accum_out=s_t[:],
            )
            scalar.drain().then_inc(asem, 1)
            # dense conf^2 over this core's slab (channel 4 of 5, stride-5
            # read) — runs on ACT in parallel with the DVE row pipeline.
            scalar.wait_ge(dsem, 32)        # slab landed
            scalar.activation(
                out=slabjunk[:], in_=slab_t[:, :, 4:5],
                func=mybir.ActivationFunctionType.Square,
                accum_out=outtile[0:SLAB_P, 0:1],
            )
            scalar.drain().then_inc(csem, 1)

    st.close()
    return nc


def _get_nc():
    if "nc" not in _STATE:
        _STATE["nc"] = _build_program()
    return _STATE["nc"]


def _build_targets_host(bbox_target):
    """Replicate reference build_targets' cell assignment exactly (jax-CPU),
    returning the winning target index per grid cell (-1 = no object)."""
    import jax
    import jax.numpy as jnp

    cpu = jax.devices("cpu")[0]
    with jax.default_device(cpu):
        bt = jnp.asarray(np.asarray(bbox_target), dtype=jnp.float32)
        gt = bt[..., :4]
        valid = jnp.sum(bt, axis=-1) != 0
        gi = (gt[..., 0] * G).astype(jnp.int32)
        gj = (gt[..., 1] * G).astype(jnp.int32)
        acx = (0.5 + gi.astype(gt.dtype)) / G
        acy = (0.5 + gj.astype(gt.dtype)) / G
        aw = jnp.asarray(ANCHORS)[:, 0]
        ah = jnp.asarray(ANCHORS)[:, 1]

        def corners(cx, cy, w, h):
            x1 = (cx - w / 2) * IMAGE_SIZE
            x2 = (cx + w / 2) * IMAGE_SIZE
            y1 = (cy - h / 2) * IMAGE_SIZE
            y2 = (cy + h / 2) * IMAGE_SIZE
            return x1, x2, y1, y2

        gx1, gx2, gy1, gy2 = corners(gt[..., 0], gt[..., 1], gt[..., 2], gt[..., 3])
        ax1, ax2, ay1, ay2 = corners(acx[..., None], acy[..., None], aw, ah)
        ix1 = jnp.maximum(gx1[..., None], ax1)
        iy1 = jnp.maximum(gy1[..., None], ay1)
        ix2 = jnp.minimum(gx2[..., None], ax2)
        iy2 = jnp.minimum(gy2[..., None], ay2)
        inter = (ix2 - ix1 + 1) * (iy2 - iy1 + 1)
        area_g = ((gx2 - gx1 + 1) * (gy2 - gy1 + 1))[..., None]
        area_a = (ax2 - ax1 + 1) * (ay2 - ay1 + 1)
        iou = inter / (area_g + area_a - inter + 1e-16)
        best = jnp.argmax(iou, axis=-1)
        b_idx = jnp.broadcast_to(jnp.arange(B)[:, None], (B, T))
        gj_s = jnp.where(valid, gj, G)
        tnum = jnp.broadcast_to(jnp.arange(T)[None, :], (B, T))
        win = (
            jnp.full((B, G, G, A), -1, jnp.int32)
            .at[b_idx, gj_s, gi, best]
            .set(tnum, mode="drop")
        )
    return np.asarray(win)


def _prepare(bbox_prediction, landmarks_prediction, bbox_target, landmarks_target):
    """Host prep: target assignment + gather.  Returns (in_maps, n_obj)."""
    bbox_prediction = np.asarray(bbox_prediction, dtype=np.float32)
    landmarks_prediction = np.asarray(landmarks_prediction, dtype=np.float32)
    bbox_target = np.asarray(bbox_target, dtype=np.float32)
    landmarks_target = np.asarray(landmarks_target, dtype=np.float32)

    win = _build_targets_host(bbox_target)
    cells = np.argwhere(win >= 0)                      # (n, 4): b, gj, gi, a
    twin = win[win >= 0]                               # aligned winners
    n_obj = len(cells)

    cb, cj, ci, ca = cells[:, 0], cells[:, 1], cells[:, 2], cells[:, 3]
    lmp_all = landmarks_prediction[cb, cj, ci, ca].reshape(n_obj, 136)
    lmt_all = landmarks_target[cb, twin].reshape(n_obj, 136)
    bbp_all = bbox_prediction[cb, cj, ci, ca, :4]      # (n, 4)
    bbt_all = np.log1p(bbox_target[cb, twin, :4]).astype(np.float32)
    conf_all = bbox_prediction[cb, cj, ci, ca, 4]      # (n,)
    nf_all = np.sqrt(bbt_all[:, 2] * bbt_all[:, 3]).astype(np.float32)
    w_all = (np.float32(1.0) / nf_all).astype(np.float32)

    in_maps = []
    for c in range(NCORES):
        sel = (cb >= c * BPC) & (cb < (c + 1) * BPC)
        r = int(sel.sum())
        small = np.zeros((ROWS, SMALL_F), np.float32)
        small[:r, 0:136] = lmp_all[sel]
        small[:r, 136:272] = lmt_all[sel]
        small[:r, 272:276] = bbp_all[sel]
        small[:r, 276:280] = bbt_all[sel]
        small[:r, 280] = conf_all[sel]
        small[:r, 281] = w_all[sel]
        small[:r, 282] = 1.0
        slab = np.ascontiguousarray(
            bbox_prediction[c * BPC:(c + 1) * BPC].reshape(SLAB_P, SLAB_F, 5))
        in_maps.append({"small": small, "slab": slab})
    return in_maps, n_obj


def _combine(results, n_obj):
    S = np.zeros(5, np.float64)
    for r in results:
        o = r["out"].astype(np.float64)
        S += o[:, :5].sum(axis=0)
    s_slab, s_nme, s_loc, s_cse, s_csq = S
    n_obj_c = max(float(n_obj), 1.0)
    n_noobj = max(float(B * CELLS - n_obj), 1.0)
    nme = 2.0 * s_nme / (68.0 * n_obj_c)
    loc = 5.0 * s_loc / (n_obj_c * 4.0)
    conf = 0.5 * (s_slab - s_csq) / n_noobj + s_cse / n_obj_c
    return (np.float32(nme), np.float32(loc), np.float32(conf))


def _run_device(in_maps, trace=False):
    from concourse.bass_utils import run_bass_kernel_spmd
    nc = _get_nc()
    return run_bass_kernel_spmd(nc, in_maps, list(range(NCORES)), trace=trace)


def kernel(bbox_prediction, landmarks_prediction, bbox_target, landmarks_target):
    in_maps, n_obj = _prepare(
        bbox_prediction, landmarks_prediction, bbox_target, landmarks_target)
    res = _run_device(in_maps)
    return _combine(res.results, n_obj)


# revision 14
# speedup vs baseline: 1.0446x; 1.0314x over previous
"""JointLoss (YOLO-style bbox + landmarks + confidence) on 8 Trainium2 cores.

Strategy: the three losses only read predictions at obj cells (<= B*T = 1024
of the 207360 grid cells) except the confidence term, which needs
sum(conf^2) over the whole grid.  Host builds the target assignment (tiny:
32x32 IoU argmax + scatter, replicated bit-exactly with jax-CPU), gathers
the obj-cell rows, and ships per-core: the core's bbox-prediction slab (for
the dense conf reduction) + gathered rows packed into one tensor.  Device
(data-parallel over batch, 4 batches/core) computes per-partition partial
sums; host combines in f64.

Raw Bass (no TileContext: its multi-wait tail drain does not compile on
this walrus build).  Explicit semaphores; DVE write-buffer drains between
dependent op levels (raw Bass does not auto-insert them).
"""

import numpy as np

B, T, G, A = 32, 32, 36, 5
NCORES = 8
BPC = B // NCORES            # batches per core
CELLS = G * G * A            # 6480 per batch
ROWS = BPC * T               # max obj rows per core = 128
SLAB_P, SLAB_F = 120, 216    # 120 * 216 * 5 == BPC * CELLS * 5 == 129600
CONF_F = 204                 # ceil(BPC*CELLS/128): conf channel, zero-padded
SMALL_F = 284 + CONF_F       # 136 lmp + 136 lmt + 4 bbp + 4 bbt + 4 aux + conf

IMAGE_SIZE = 288.0
ANCHORS = np.array([[0.24, 0.24], [0.12, 0.12], [0.08, 0.08],
                    [0.28, 0.28], [0.15, 0.15]], dtype=np.float32)

_STATE = {}


def _build_program():
    import concourse.bass as bass
    from concourse import mybir
    from contextlib import ExitStack

    nc = bass.Bass()
    f32 = mybir.dt.float32
    small_p = nc.declare_dram_parameter("small", [ROWS, SMALL_F], f32, isOutput=False)
    out_p = nc.declare_dram_parameter("out", [ROWS, 8], f32, isOutput=True)

    st = ExitStack()
    Tt = lambda n, s: st.enter_context(nc.sbuf_tensor(n, s, f32))
    small_t = Tt("small_t", [ROWS, SMALL_F])
    slabjunk = Tt("slabjunk", [ROWS, CONF_F])
    ldiff = Tt("ldiff", [ROWS, 68, 2])
    lsq = Tt("lsq", [ROWS, 68, 2])
    pairsum = Tt("pairsum", [ROWS, 68, 1])
    d_t = Tt("d_t", [ROWS, 68, 1])
    s_t = Tt("s_t", [ROWS, 1])
    bdiff = Tt("bdiff", [ROWS, 4])
    bneg = Tt("bneg", [ROWS, 4])
    bad = Tt("bad", [ROWS, 4])
    bt_ = Tt("bt_", [ROWS, 4])
    bth = Tt("bth", [ROWS, 4])
    bu = Tt("bu", [ROWS, 4])
    bsl = Tt("bsl", [ROWS, 4])
    ones4 = Tt("ones4", [ROWS, 4])
    negh4 = Tt("negh4", [ROWS, 4])
    zero4 = Tt("zero4", [ROWS, 4])
    cm1 = Tt("cm1", [ROWS, 1])
    cm1sq = Tt("cm1sq", [ROWS, 1])
    csq_ = Tt("csq_", [ROWS, 1])
    outtile = Tt("outtile", [ROWS, 8])

    lmp_v = small_t[:, 0:136]
    lmt_v = small_t[:, 136:272]
    bbp_v = small_t[:, 272:276]
    bbt_v = small_t[:, 276:280]
    aux0 = small_t[:, 280:281]   # gathered conf
    aux1 = small_t[:, 281:282]   # mask / nf
    aux2 = small_t[:, 282:283]   # mask
    conf_v = small_t[:, 284:284 + CONF_F]   # zero-padded dense conf channel

    op = mybir.AluOpType
    ax = mybir.AxisListType

    with nc.Block() as block, \
            nc.semaphore("dsem") as dsem, \
            nc.semaphore("vsem") as vsem, \
            nc.semaphore("asem") as asem, \
            nc.semaphore("csem") as csem, \
            nc.semaphore("msem") as msem, \
            nc.semaphore("osem") as osem:

        @block.sync
        def _(sync):
            sync.dma_start(out=small_t[:], in_=small_p[:]).then_inc(dsem, 16)
            sync.wait_ge(csem, 2)
            sync.dma_start(out=out_p[:], in_=outtile[:]).then_inc(osem, 16)
            sync.wait_ge(osem, 16)

        @block.vector
        def _(vector):
            vector.memset(outtile[:], 0.0)
            vector.memset(ones4[:], 1.0)
            vector.memset(negh4[:], -0.5)
            vector.memset(zero4[:], 0.0)
            vector.drain().then_inc(msem, 1)
            vector.wait_ge(dsem, 16)        # small rows landed
            # L1
            vector.tensor_tensor(out=ldiff[:], in0=lmp_v, in1=lmt_v, op=op.subtract)
            vector.tensor_tensor(out=bdiff[:], in0=bbp_v, in1=bbt_v, op=op.subtract)
            vector.tensor_tensor(out=cm1[:], in0=aux0, in1=ones4[:, 0:1], op=op.subtract)
            vector.tensor_tensor(out=csq_[:], in0=aux0, in1=aux0, op=op.mult)
            vector.drain()
            # L2
            vector.tensor_mul(lsq[:], ldiff[:], ldiff[:])
            vector.tensor_sub(bneg[:], zero4[:], bdiff[:])
            vector.tensor_mul(cm1sq[:], cm1[:], cm1[:])
            vector.tensor_mul(outtile[:, 4:5], csq_[:], aux2)
            vector.drain()
            # L3
            vector.tensor_tensor(out=pairsum[:], in0=lsq[:, :, 0:1], in1=lsq[:, :, 1:2], op=op.add)
            vector.tensor_tensor(out=bad[:], in0=bdiff[:], in1=bneg[:], op=op.max)
            vector.tensor_mul(outtile[:, 3:4], cm1sq[:], aux2)
            vector.drain().then_inc(vsem, 1)      # ACT may start sqrt
            # smooth-L1 tail: t=min(|d|,1); sl1 = t*(|d| - 0.5 t)
            vector.tensor_tensor(out=bt_[:], in0=bad[:], in1=ones4[:], op=op.min)
            vector.drain()
            vector.tensor_mul(bth[:], bt_[:], negh4[:])
            vector.drain()
            vector.tensor_add(bu[:], bad[:], bth[:])
            vector.drain()
            vector.tensor_mul(bsl[:], bt_[:], bu[:])
            vector.drain()
            vector.tensor_reduce(out=outtile[:, 2:3], in_=bsl[:], axis=ax.X, op=op.add)
            # nme = s * (mask/nf)
            vector.wait_ge(asem, 1)
            vector.tensor_mul(outtile[:, 1:2], s_t[:], aux1)
            vector.drain().then_inc(csem, 1)

        @block.scalar
        def _(scalar):
            # dense conf^2 (host pre-extracted channel, zero-padded) — runs
            # on ACT in parallel with the DVE row pipeline, before the sqrt.
            scalar.wait_ge(msem, 1)         # outtile memset drained
            scalar.wait_ge(dsem, 16)
            scalar.activation(
                out=slabjunk[:], in_=conf_v,
                func=mybir.ActivationFunctionType.Square,
                accum_out=outtile[:, 0:1],
            )
            scalar.drain().then_inc(csem, 1)
            # landmark distances: d = sqrt(dx^2+dy^2), s = sum_l d
            scalar.wait_ge(vsem, 1)
            scalar.activation(
                out=d_t[:], in_=pairsum[:],
                func=mybir.ActivationFunctionType.Sqrt, accum_out=s_t[:],
            )
            scalar.drain().then_inc(asem, 1)

    st.close()
    return nc


def _get_nc():
    if "nc" not in _STATE:
        _STATE["nc"] = _build_program()
    return _STATE["nc"]


def _build_targets_host(bbox_target):
    """Replicate reference build_targets' cell assignment exactly (jax-CPU),
    returning the winning target index per grid cell (-1 = no object)."""
    import jax
    import jax.numpy as jnp

    cpu = jax.devices("cpu")[0]
    with jax.default_device(cpu):
        bt = jnp.asarray(np.asarray(bbox_target), dtype=jnp.float32)
        gt = bt[..., :4]
        valid = jnp.sum(bt, axis=-1) != 0
        gi = (gt[..., 0] * G).astype(jnp.int32)
        gj = (gt[..., 1] * G).astype(jnp.int32)
        acx = (0.5 + gi.astype(gt.dtype)) / G
        acy = (0.5 + gj.astype(gt.dtype)) / G
        aw = jnp.asarray(ANCHORS)[:, 0]
        ah = jnp.asarray(ANCHORS)[:, 1]

        def corners(cx, cy, w, h):
            x1 = (cx - w / 2) * IMAGE_SIZE
            x2 = (cx + w / 2) * IMAGE_SIZE
            y1 = (cy - h / 2) * IMAGE_SIZE
            y2 = (cy + h / 2) * IMAGE_SIZE
            return x1, x2, y1, y2

        gx1, gx2, gy1, gy2 = corners(gt[..., 0], gt[..., 1], gt[..., 2], gt[..., 3])
        ax1, ax2, ay1, ay2 = corners(acx[..., None], acy[..., None], aw, ah)
        ix1 = jnp.maximum(gx1[..., None], ax1)
        iy1 = jnp.maximum(gy1[..., None], ay1)
        ix2 = jnp.minimum(gx2[..., None], ax2)
        iy2 = jnp.minimum(gy2[..., None], ay2)
        inter = (ix2 - ix1 + 1) * (iy2 - iy1 + 1)
        area_g = ((gx2 - gx1 + 1) * (gy2 - gy1 + 1))[..., None]
        area_a = (ax2 - ax1 + 1) * (ay2 - ay1 + 1)
        iou = inter / (area_g + area_a - inter + 1e-16)
        best = jnp.argmax(iou, axis=-1)
        b_idx = jnp.broadcast_to(jnp.arange(B)[:, None], (B, T))
        gj_s = jnp.where(valid, gj, G)
        tnum = jnp.broadcast_to(jnp.arange(T)[None, :], (B, T))
        win = (
            jnp.full((B, G, G, A), -1, jnp.int32)
            .at[b_idx, gj_s, gi, best]
            .set(tnum, mode="drop")
        )
    return np.asarray(win)


def _prepare(bbox_prediction, landmarks_prediction, bbox_target, landmarks_target):
    """Host prep: target assignment + gather.  Returns (in_maps, n_obj)."""
    bbox_prediction = np.asarray(bbox_prediction, dtype=np.float32)
    landmarks_prediction = np.asarray(landmarks_prediction, dtype=np.float32)
    bbox_target = np.asarray(bbox_target, dtype=np.float32)
    landmarks_target = np.asarray(landmarks_target, dtype=np.float32)

    win = _build_targets_host(bbox_target)
    cells = np.argwhere(win >= 0)                      # (n, 4): b, gj, gi, a
    twin = win[win >= 0]                               # aligned winners
    n_obj = len(cells)

    cb, cj, ci, ca = cells[:, 0], cells[:, 1], cells[:, 2], cells[:, 3]
    lmp_all = landmarks_prediction[cb, cj, ci, ca].reshape(n_obj, 136)
    lmt_all = landmarks_target[cb, twin].reshape(n_obj, 136)
    bbp_all = bbox_prediction[cb, cj, ci, ca, :4]      # (n, 4)
    bbt_all = np.log1p(bbox_target[cb, twin, :4]).astype(np.float32)
    conf_all = bbox_prediction[cb, cj, ci, ca, 4]      # (n,)
    nf_all = np.sqrt(bbt_all[:, 2] * bbt_all[:, 3]).astype(np.float32)
    w_all = (np.float32(1.0) / nf_all).astype(np.float32)

    in_maps = []
    for c in range(NCORES):
        sel = (cb >= c * BPC) & (cb < (c + 1) * BPC)
        r = int(sel.sum())
        small = np.zeros((ROWS, SMALL_F), np.float32)
        small[:r, 0:136] = lmp_all[sel]
        small[:r, 136:272] = lmt_all[sel]
        small[:r, 272:276] = bbp_all[sel]
        small[:r, 276:280] = bbt_all[sel]
        small[:r, 280] = conf_all[sel]
        small[:r, 281] = w_all[sel]
        small[:r, 282] = 1.0
        confc = bbox_prediction[c * BPC:(c + 1) * BPC, :, :, :, 4].reshape(-1)
        conf_pad = np.zeros(ROWS * CONF_F, np.float32)
        conf_pad[:confc.size] = confc
        small[:, 284:] = conf_pad.reshape(ROWS, CONF_F)
        in_maps.append({"small": small})
    return in_maps, n_obj


def _combine(results, n_obj):
    S = np.zeros(5, np.float64)
    for r in results:
        o = r["out"].astype(np.float64)
        S += o[:, :5].sum(axis=0)
    s_slab, s_nme, s_loc, s_cse, s_csq = S
    n_obj_c = max(float(n_obj), 1.0)
    n_noobj = max(float(B * CELLS - n_obj), 1.0)
    nme = 2.0 * s_nme / (68.0 * n_obj_c)
    loc = 5.0 * s_loc / (n_obj_c * 4.0)
    conf = 0.5 * (s_slab - s_csq) / n_noobj + s_cse / n_obj_c
    return (np.float32(nme), np.float32(loc), np.float32(conf))


def _run_device(in_maps, trace=False):
    from concourse.bass_utils import run_bass_kernel_spmd
    nc = _get_nc()
    return run_bass_kernel_spmd(nc, in_maps, list(range(NCORES)), trace=trace)


def kernel(bbox_prediction, landmarks_prediction, bbox_target, landmarks_target):
    in_maps, n_obj = _prepare(
        bbox_prediction, landmarks_prediction, bbox_target, landmarks_target)
    res = _run_device(in_maps)
    return _combine(res.results, n_obj)


# revision 15
# speedup vs baseline: 1.0599x; 1.0146x over previous
"""JointLoss (YOLO-style bbox + landmarks + confidence) on 8 Trainium2 cores.

Strategy: the three losses only read predictions at obj cells (<= B*T = 1024
of the 207360 grid cells) except the confidence term, which needs
sum(conf^2) over the whole grid.  Host builds the target assignment (tiny:
32x32 IoU argmax + scatter, replicated bit-exactly with jax-CPU), gathers
the obj-cell rows, and ships per-core: the core's bbox-prediction slab (for
the dense conf reduction) + gathered rows packed into one tensor.  Device
(data-parallel over batch, 4 batches/core) computes per-partition partial
sums; host combines in f64.

Raw Bass (no TileContext: its multi-wait tail drain does not compile on
this walrus build).  Explicit semaphores; DVE write-buffer drains between
dependent op levels (raw Bass does not auto-insert them).
"""

import numpy as np

B, T, G, A = 32, 32, 36, 5
NCORES = 8
BPC = B // NCORES            # batches per core
CELLS = G * G * A            # 6480 per batch
ROWS = BPC * T               # max obj rows per core = 128
SLAB_P, SLAB_F = 120, 216    # 120 * 216 * 5 == BPC * CELLS * 5 == 129600
CONF_F = 204                 # ceil(BPC*CELLS/128): conf channel, zero-padded
SMALL_F = 284 + CONF_F       # 136 lmp + 136 lmt + 4 bbp + 4 bbt + 4 aux + conf

IMAGE_SIZE = 288.0
ANCHORS = np.array([[0.24, 0.24], [0.12, 0.12], [0.08, 0.08],
                    [0.28, 0.28], [0.15, 0.15]], dtype=np.float32)

_STATE = {}


def _build_program():
    import concourse.bass as bass
    from concourse import mybir
    from contextlib import ExitStack

    nc = bass.Bass()
    f32 = mybir.dt.float32
    small_p = nc.declare_dram_parameter("small", [ROWS, SMALL_F], f32, isOutput=False)
    out_p = nc.declare_dram_parameter("out", [ROWS, 8], f32, isOutput=True)

    st = ExitStack()
    Tt = lambda n, s: st.enter_context(nc.sbuf_tensor(n, s, f32))
    small_t = Tt("small_t", [ROWS, SMALL_F])
    slabjunk = Tt("slabjunk", [ROWS, CONF_F])
    ldiff = Tt("ldiff", [ROWS, 68, 2])
    lsq = Tt("lsq", [ROWS, 68, 2])
    pairsum = Tt("pairsum", [ROWS, 68, 1])
    d_t = Tt("d_t", [ROWS, 68, 1])
    s_t = Tt("s_t", [ROWS, 1])
    bdiff = Tt("bdiff", [ROWS, 4])
    bneg = Tt("bneg", [ROWS, 4])
    bad = Tt("bad", [ROWS, 4])
    bt_ = Tt("bt_", [ROWS, 4])
    bth = Tt("bth", [ROWS, 4])
    bu = Tt("bu", [ROWS, 4])
    bsl = Tt("bsl", [ROWS, 4])
    ones4 = Tt("ones4", [ROWS, 4])
    negh4 = Tt("negh4", [ROWS, 4])
    zero4 = Tt("zero4", [ROWS, 4])
    cm1 = Tt("cm1", [ROWS, 1])
    cm1sq = Tt("cm1sq", [ROWS, 1])
    csq_ = Tt("csq_", [ROWS, 1])
    outtile = Tt("outtile", [ROWS, 8])

    lmp_v = small_t[:, 0:136]
    lmt_v = small_t[:, 136:272]
    bbp_v = small_t[:, 272:276]
    bbt_v = small_t[:, 276:280]
    aux0 = small_t[:, 280:281]   # gathered conf
    aux1 = small_t[:, 281:282]   # (mask / nf)^2  (folded into the ACT sqrt scale)
    aux2 = small_t[:, 282:283]   # mask
    conf_v = small_t[:, 284:284 + CONF_F]   # zero-padded dense conf channel

    op = mybir.AluOpType
    ax = mybir.AxisListType

    with nc.Block() as block, \
            nc.semaphore("dsem") as dsem, \
            nc.semaphore("vsem") as vsem, \
            nc.semaphore("csem") as csem, \
            nc.semaphore("msem") as msem, \
            nc.semaphore("osem") as osem:

        @block.sync
        def _(sync):
            sync.dma_start(out=small_t[:], in_=small_p[:]).then_inc(dsem, 16)
            sync.wait_ge(csem, 3)
            sync.dma_start(out=out_p[:], in_=outtile[:]).then_inc(osem, 16)
            sync.wait_ge(osem, 16)

        @block.vector
        def _(vector):
            vector.memset(outtile[:], 0.0)
            vector.memset(ones4[:], 1.0)
            vector.memset(negh4[:], -0.5)
            vector.memset(zero4[:], 0.0)
            vector.drain().then_inc(msem, 1)
            vector.wait_ge(dsem, 16)        # small rows landed
            # L1
            vector.tensor_tensor(out=ldiff[:], in0=lmp_v, in1=lmt_v, op=op.subtract)
            vector.tensor_tensor(out=bdiff[:], in0=bbp_v, in1=bbt_v, op=op.subtract)
            vector.tensor_tensor(out=cm1[:], in0=aux0, in1=ones4[:, 0:1], op=op.subtract)
            vector.tensor_tensor(out=csq_[:], in0=aux0, in1=aux0, op=op.mult)
            vector.drain()
            # L2
            vector.tensor_mul(lsq[:], ldiff[:], ldiff[:])
            vector.tensor_sub(bneg[:], zero4[:], bdiff[:])
            vector.tensor_mul(cm1sq[:], cm1[:], cm1[:])
            vector.tensor_mul(outtile[:, 4:5], csq_[:], aux2)
            vector.drain()
            # L3
            vector.tensor_tensor(out=pairsum[:], in0=lsq[:, :, 0:1], in1=lsq[:, :, 1:2], op=op.add)
            vector.tensor_tensor(out=bad[:], in0=bdiff[:], in1=bneg[:], op=op.max)
            vector.tensor_mul(outtile[:, 3:4], cm1sq[:], aux2)
            vector.drain().then_inc(vsem, 1)      # ACT may start sqrt
            # smooth-L1 tail: t=min(|d|,1); sl1 = t*(|d| - 0.5 t)
            vector.tensor_tensor(out=bt_[:], in0=bad[:], in1=ones4[:], op=op.min)
            vector.drain()
            vector.tensor_mul(bth[:], bt_[:], negh4[:])
            vector.drain()
            vector.tensor_add(bu[:], bad[:], bth[:])
            vector.drain()
            vector.tensor_mul(bsl[:], bt_[:], bu[:])
            vector.drain()
            vector.tensor_reduce(out=outtile[:, 2:3], in_=bsl[:], axis=ax.X, op=op.add)
            vector.drain().then_inc(csem, 1)

        @block.scalar
        def _(scalar):
            # dense conf^2 (host pre-extracted channel, zero-padded) — runs
            # on ACT in parallel with the DVE row pipeline, before the sqrt.
            scalar.wait_ge(msem, 1)         # outtile memset drained
            scalar.wait_ge(dsem, 16)
            scalar.activation(
                out=slabjunk[:], in_=conf_v,
                func=mybir.ActivationFunctionType.Square,
                accum_out=outtile[:, 0:1],
            )
            scalar.drain().then_inc(csem, 1)
            # weighted landmark distances in one op:
            # sqrt(pairsum * w^2) = w * sqrt(dx^2+dy^2);  accum -> nme partials
            scalar.wait_ge(vsem, 1)
            scalar.activation(
                out=d_t[:], in_=pairsum[:],
                func=mybir.ActivationFunctionType.Sqrt, scale=aux1,
                accum_out=outtile[:, 1:2],
            )
            scalar.drain().then_inc(csem, 1)

    st.close()
    return nc


def _get_nc():
    if "nc" not in _STATE:
        _STATE["nc"] = _build_program()
    return _STATE["nc"]


def _build_targets_host(bbox_target):
    """Replicate reference build_targets' cell assignment exactly (jax-CPU),
    returning the winning target index per grid cell (-1 = no object)."""
    import jax
    import jax.numpy as jnp

    cpu = jax.devices("cpu")[0]
    with jax.default_device(cpu):
        bt = jnp.asarray(np.asarray(bbox_target), dtype=jnp.float32)
        gt = bt[..., :4]
        valid = jnp.sum(bt, axis=-1) != 0
        gi = (gt[..., 0] * G).astype(jnp.int32)
        gj = (gt[..., 1] * G).astype(jnp.int32)
        acx = (0.5 + gi.astype(gt.dtype)) / G
        acy = (0.5 + gj.astype(gt.dtype)) / G
        aw = jnp.asarray(ANCHORS)[:, 0]
        ah = jnp.asarray(ANCHORS)[:, 1]

        def corners(cx, cy, w, h):
            x1 = (cx - w / 2) * IMAGE_SIZE
            x2 = (cx + w / 2) * IMAGE_SIZE
            y1 = (cy - h / 2) * IMAGE_SIZE
            y2 = (cy + h / 2) * IMAGE_SIZE
            return x1, x2, y1, y2

        gx1, gx2, gy1, gy2 = corners(gt[..., 0], gt[..., 1], gt[..., 2], gt[..., 3])
        ax1, ax2, ay1, ay2 = corners(acx[..., None], acy[..., None], aw, ah)
        ix1 = jnp.maximum(gx1[..., None], ax1)
        iy1 = jnp.maximum(gy1[..., None], ay1)
        ix2 = jnp.minimum(gx2[..., None], ax2)
        iy2 = jnp.minimum(gy2[..., None], ay2)
        inter = (ix2 - ix1 + 1) * (iy2 - iy1 + 1)
        area_g = ((gx2 - gx1 + 1) * (gy2 - gy1 + 1))[..., None]
        area_a = (ax2 - ax1 + 1) * (ay2 - ay1 + 1)
        iou = inter / (area_g + area_a - inter + 1e-16)
        best = jnp.argmax(iou, axis=-1)
        b_idx = jnp.broadcast_to(jnp.arange(B)[:, None], (B, T))
        gj_s = jnp.where(valid, gj, G)
        tnum = jnp.broadcast_to(jnp.arange(T)[None, :], (B, T))
        win = (
            jnp.full((B, G, G, A), -1, jnp.int32)
            .at[b_idx, gj_s, gi, best]
            .set(tnum, mode="drop")
        )
    return np.asarray(win)


def _prepare(bbox_prediction, landmarks_prediction, bbox_target, landmarks_target):
    """Host prep: target assignment + gather.  Returns (in_maps, n_obj)."""
    bbox_prediction = np.asarray(bbox_prediction, dtype=np.float32)
    landmarks_prediction = np.asarray(landmarks_prediction, dtype=np.float32)
    bbox_target = np.asarray(bbox_target, dtype=np.float32)
    landmarks_target = np.asarray(landmarks_target, dtype=np.float32)

    win = _build_targets_host(bbox_target)
    cells = np.argwhere(win >= 0)                      # (n, 4): b, gj, gi, a
    twin = win[win >= 0]                               # aligned winners
    n_obj = len(cells)

    cb, cj, ci, ca = cells[:, 0], cells[:, 1], cells[:, 2], cells[:, 3]
    lmp_all = landmarks_prediction[cb, cj, ci, ca].reshape(n_obj, 136)
    lmt_all = landmarks_target[cb, twin].reshape(n_obj, 136)
    bbp_all = bbox_prediction[cb, cj, ci, ca, :4]      # (n, 4)
    bbt_all = np.log1p(bbox_target[cb, twin, :4]).astype(np.float32)
    conf_all = bbox_prediction[cb, cj, ci, ca, 4]      # (n,)
    nf_all = np.sqrt(bbt_all[:, 2] * bbt_all[:, 3]).astype(np.float32)
    w_all = (np.float32(1.0) / nf_all).astype(np.float32)

    in_maps = []
    for c in range(NCORES):
        sel = (cb >= c * BPC) & (cb < (c + 1) * BPC)
        r = int(sel.sum())
        small = np.zeros((ROWS, SMALL_F), np.float32)
        small[:r, 0:136] = lmp_all[sel]
        small[:r, 136:272] = lmt_all[sel]
        small[:r, 272:276] = bbp_all[sel]
        small[:r, 276:280] = bbt_all[sel]
        small[:r, 280] = conf_all[sel]
        small[:r, 281] = (w_all * w_all)[sel]
        small[:r, 282] = 1.0
        confc = bbox_prediction[c * BPC:(c + 1) * BPC, :, :, :, 4].reshape(-1)
        conf_pad = np.zeros(ROWS * CONF_F, np.float32)
        conf_pad[:confc.size] = confc
        small[:, 284:] = conf_pad.reshape(ROWS, CONF_F)
        in_maps.append({"small": small})
    return in_maps, n_obj


def _combine(results, n_obj):
    S = np.zeros(5, np.float64)
    for r in results:
        o = r["out"].astype(np.float64)
        S += o[:, :5].sum(axis=0)
    s_slab, s_nme, s_loc, s_cse, s_csq = S
    n_obj_c = max(float(n_obj), 1.0)
    n_noobj = max(float(B * CELLS - n_obj), 1.0)
    nme = 2.0 * s_nme / (68.0 * n_obj_c)
    loc = 5.0 * s_loc / (n_obj_c * 4.0)
    conf = 0.5 * (s_slab - s_csq) / n_noobj + s_cse / n_obj_c
    return (np.float32(nme), np.float32(loc), np.float32(conf))


def _run_device(in_maps, trace=False):
    from concourse.bass_utils import run_bass_kernel_spmd
    nc = _get_nc()
    return run_bass_kernel_spmd(nc, in_maps, list(range(NCORES)), trace=trace)


def kernel(bbox_prediction, landmarks_prediction, bbox_target, landmarks_target):
    in_maps, n_obj = _prepare(
        bbox_prediction, landmarks_prediction, bbox_target, landmarks_target)
    res = _run_device(in_maps)
    return _combine(res.results, n_obj)
